# revision 72
# baseline (speedup 1.0000x reference)
"""Fused cross-entropy loss over a 100k item vocabulary on 8 Trainium2 cores.

Math (matches the reference):
    logits = hidden_flat @ item_emb.T          # [n_tok, 100000]
    nll[r] = log(sum_v exp(logits[r, v])) - logits[r, label[r]]
    loss   = mean over ACTIVE tokens of nll

Key optimizations over a straight implementation:

1. Active-row compaction (host side): only the ~half of token rows that are
   active (attention mask past the prompt, next-token shift) contribute to
   the loss, so softmax denominators are computed only for those rows,
   gathered into NB=ceil(n_active/128) blocks of 128. Halves all device work.

2. Vocab tensor-parallel over 8 cores (12500 columns each) with fp8-e4m3
   DoubleRow matmuls (fp32 PSUM accumulate; emb pre-scaled x32 on the host).

3. Three-engine exp+row-sum. The per-core [NB*128, 12500] exp()+sum work is
   split into two vocab regions so PE, ACT and DVE all run near roofline:
     - A-region (tokens on partitions): ACT exp in place in PSUM with fused
       accumulated row-sum (accum_out).
     - B-region (vocab on partitions): DVE computes a Schraudolph-style fast
       exp via an int8 bit trick - int8(A*psum + B) IS the fp8-e4m3 bit
       pattern of ~exp(logit) - and the vocab-dim reduction is done by cheap
       PE DoubleRow ones-matmuls accumulating over all vocab pairs. The bias
       constant is tuned so the approximation is unbiased over the logit
       distribution; residual sawtooth noise (~6% per element) averages out
       over the ~39k summed terms per denominator (<0.1% on ln S).

4. Label logits ride the same PE machinery: the label embeddings are packed
   as two extra fp8 DoubleRow pairs, and each block's diagonal is extracted
   from the resulting 128x128 PSUM tiles with a (1/scale)-scaled identity
   multiply + row reduce on DVE. No extra DMA streams or DVE dot loops.

A 2KB AllGather combines per-core partial denominators; every core finishes
the masked-mean loss locally (core 0's value is returned).
"""
import sys

try:
    import concourse.bass as _cb  # provided by the environment boot path
except ModuleNotFoundError:
    sys.path.insert(0, "/opt/trn_rl_repo")

import numpy as np

import concourse.bass as bass
import concourse.bacc as bacc
import concourse.tile as tile
import concourse.mybir as mybir
from concourse import bass_utils

# Force Exp and Ln to resolve to one activation-function table set (the
# act_info set containing both) so the epilogue Ln does not pay a 1.3us
# ACT table reload on the critical tail. Indices into act_info.json are
# preserved; only membership visibility to the table-choice pass changes.
import concourse.hw_specs as _hw_specs
import concourse.bacc as _bacc_mod

_orig_get_tables = _hw_specs.get_activation_tables


def _patched_get_tables(arch):
    tabs = dict(_orig_get_tables(arch))
    AF = mybir.ActivationFunctionType
    both = [n for n, s in tabs.items() if AF.Exp in s and AF.Ln in s]
    if both:
        keep = set(both)
        tabs = {
            n: (s if n in keep else (set(s) - {AF.Exp, AF.Ln}))
            for n, s in tabs.items()
        }
    return tabs


_bacc_mod.get_activation_tables = _patched_get_tables

N_CORES = 8
B, L, D = 8, 128, 768
V = 100000
VS = V // N_CORES            # vocab shard per core
KC2 = D // 256               # DoubleRow contraction chunks
NUM_USERS = 10000
LABEL_OFFSET = 151669 + NUM_USERS

BF16 = mybir.dt.bfloat16
F32 = mybir.dt.float32
FP8 = mybir.dt.float8e4
I8 = mybir.dt.int8
NP_BF16 = mybir.dt.np(BF16)
NP_FP8 = mybir.dt.np(FP8)

EMB_SCALE = 32.0
LOG2E = 1.4426950408889634

# ---- per-core vocab split: A-region (ACT lane) | B-region (DVE+PE lane) ----
VB = 4864                    # B-region width, multiple of 256
NPAIRS = VB // 256
VA = VS - VB
# A chunk widths; chunk ci lives in PSUM slot ci%2 (bank budget per slot below)
A_SLOT_W = (1536, 1024)      # PSUM A-slot widths: 3 + 2 banks
A_WIDTHS = [512, 1024, 1536, 1024, 1536, 1024, 512, 468]
assert sum(A_WIDTHS) == VA
assert all(w <= A_SLOT_W[i % 2] for i, w in enumerate(A_WIDTHS))
# A units handed to the DVE int32 fast-exp lane instead of ACT: (chunk, block)
DVEA_UNITS = []
NCHA = len(A_WIDTHS)
A_OFFS = [sum(A_WIDTHS[:i]) for i in range(NCHA)]

# how many B half-pair units to emit before the first A unit
B_HEAD = 3
B_SPAN = 0.85                # fraction of A units over which B pairs spread
B_DMA_GROUP = 1              # B pairs fetched per DMA (pair-major eTB layout)
WARMUP = 25                  # dummy PE matmuls at t=0 to climb the p-state ramp
DEBUG_OUT = False            # dump per-row intermediates (s_a, s_bt, dot) to DRAM
HT_SPLIT = False             # split hT DMA so block-0 fills start earlier
# insert aux (hpb/gpb/w/identity) DMAs after this unit index
AUX_DMA_FRAC = 0.62

# ---------------------------------------------------------------------------
# Schraudolph fast-exp bias tuning: choose d so the estimator is unbiased
# (E[approx/true] = 1) for logits ~ N(0, sigma_l).
# ---------------------------------------------------------------------------


def _fp8e4m3_decode(i):
    i = np.asarray(i, dtype=np.int64)
    e = i >> 3
    m = i & 7
    return np.where(e > 0, (1.0 + m / 8.0) * 2.0 ** (e - 7.0), (m / 8.0) * 2.0 ** -6.0)


def _tune_d8(sigma_l=0.55, n=400000):
    # deterministic normal quantile grid
    k = (np.arange(n) + 0.5) / n
    # inverse normal CDF via numpy (Acklam-style not needed: use erfinv)
    from numpy import sqrt
    try:
        from scipy.special import erfinv  # noqa: PLC0415
        z = sqrt(2.0) * erfinv(2 * k - 1)
    except Exception:
        # logistic approximation is plenty for bias tuning
        z = np.log(k / (1 - k)) / 1.702
    y = z * sigma_l * LOG2E
    true = 2.0**y

    def bias(d):
        i = np.floor(8.0 * (y + 7.0 + d) + 0.5).astype(np.int64)
        return np.mean(_fp8e4m3_decode(i) / true) - 1.0

    lo, hi = -0.15, 0.05
    for _ in range(50):
        mid = 0.5 * (lo + hi)
        if bias(mid) > 0:
            hi = mid
        else:
            lo = mid
    return 0.5 * (lo + hi)


D8 = _tune_d8()
A8_MUL = 8.0 * LOG2E / EMB_SCALE
A8_ADD = (7.0 + D8) * 8.0


def _f32_decode(i):
    i = np.asarray(i, dtype=np.int64)
    e = i >> 23
    m = i & ((1 << 23) - 1)
    return (1.0 + m * 2.0**-23.0) * 2.0 ** (e - 127.0)


def _tune_d32(sigma_l=0.55, n=400000):
    k = (np.arange(n) + 0.5) / n
    try:
        from scipy.special import erfinv  # noqa: PLC0415
        z = np.sqrt(2.0) * erfinv(2 * k - 1)
    except Exception:
        z = np.log(k / (1 - k)) / 1.702
    y = z * sigma_l * LOG2E
    true = 2.0**y
    sc = 2.0**23

    def bias(d):
        i = np.floor(sc * (y + 127.0 + d) + 0.5).astype(np.int64)
        return np.mean(_f32_decode(i) / true) - 1.0

    lo, hi = -0.15, 0.05
    for _ in range(50):
        mid = 0.5 * (lo + hi)
        if bias(mid) > 0:
            hi = mid
        else:
            lo = mid
    return 0.5 * (lo + hi)


D32 = _tune_d32()
A32_MUL = (2.0**23) * LOG2E / EMB_SCALE
A32_ADD = (127.0 + D32) * 2.0**23

_prog_cache = {}


def _unit_schedule(NB):
    """Interleave A units (chunk-pair x block round-robin) with B pairs."""
    a_units = []
    ci = 0
    while ci < NCHA:
        pair = [ci] if ci + 1 >= NCHA else [ci, ci + 1]
        for b in range(NB):
            for c in pair:
                a_units.append(("A", c, b))
        ci += 2
    # B half-pair units: (pair, token-half); B_HEAD up front, the rest spread
    # over the first ~85% of A units
    b_units = [("B", p, h) for p in range(NPAIRS) for h in range(2)]
    nbu = len(b_units)
    mixed = list(b_units[:B_HEAD])
    rest = nbu - B_HEAD
    na = len(a_units)
    span = max(1, int(na * B_SPAN))
    next_b = B_HEAD
    for ai, au in enumerate(a_units):
        mixed.append(au)
        while next_b < nbu and (next_b - B_HEAD + 1) * span <= rest * min(ai + 1, span):
            mixed.append(b_units[next_b])
            next_b += 1
    mixed.extend(b_units[next_b:])
    return mixed


def build_program(NB: int = 4, sim_single_core: bool = False):
    key = (NB, sim_single_core)
    if key in _prog_cache:
        return _prog_cache[key]
    TPAD = NB * 128

    nc = bacc.Bacc(
        "TRN2",
        target_bir_lowering=False,
        debug=False,
        enable_asserts=True,
        num_devices=1 if sim_single_core else N_CORES,
    )
    NGP = (NB + 1) // 2  # label-embedding pairs for the PE-side label dots
    hT = nc.dram_tensor("hT", [128, KC2, 2, TPAD], FP8, kind="ExternalInput")
    eT = nc.dram_tensor("eT", [128, KC2, 2, VA], FP8, kind="ExternalInput")
    eTB = nc.dram_tensor("eTB", [128, NPAIRS, KC2, 2, 256], FP8, kind="ExternalInput")
    eTG = nc.dram_tensor("eTG", [128, NGP, KC2, 2, 256], FP8, kind="ExternalInput")
    wpb = nc.dram_tensor("wpb", [128, NB], F32, kind="ExternalInput")
    idm = nc.dram_tensor("idm", [128, 128], BF16, kind="ExternalInput")
    idg = nc.dram_tensor("idg", [128, 128], BF16, kind="ExternalInput")
    loss = nc.dram_tensor("loss", [1, 3], F32, kind="ExternalOutput")
    if DEBUG_OUT:
        dbg_sa = nc.dram_tensor("dbg_sa", [128, NB], F32, kind="ExternalOutput")
        dbg_sbt = nc.dram_tensor("dbg_sbt", [128, NB], F32, kind="ExternalOutput")
        dbg_dot = nc.dram_tensor("dbg_dot", [128, NB], F32, kind="ExternalOutput")

    add = mybir.AluOpType.add
    mult = mybir.AluOpType.mult
    AF = mybir.ActivationFunctionType
    AX = mybir.AxisListType
    DR = mybir.MatmulPerfMode.DoubleRow

    mixed = _unit_schedule(NB)
    n_units = len(mixed)
    aux_at = int(n_units * AUX_DMA_FRAC)

    with tile.TileContext(nc) as tc:
        with (
            tc.tile_pool(name="const", bufs=1) as cpool,
            tc.tile_pool(name="rta", bufs=6) as rpa,
            tc.tile_pool(name="rtb", bufs=6) as rpb,
            tc.tile_pool(name="psA0", bufs=1, space="PSUM") as pa0,
            tc.tile_pool(name="psA1", bufs=1, space="PSUM") as pa1,
            tc.tile_pool(name="psB", bufs=2, space="PSUM") as pbp,
            tc.tile_pool(name="psacc", bufs=1, space="PSUM") as pacc,
            tc.tile_pool(name="dram", bufs=1, space="DRAM") as dpool,
        ):
            # ---- resident tensors -------------------------------------------
            # block-0 token slice first: unblocks the first A fills ~1us early
            ht_sb = cpool.tile([128, KC2, 2, TPAD], FP8)
            if HT_SPLIT:
                nc.sync.dma_start(ht_sb[:, :, :, 0:128], hT.ap()[:, :, :, 0:128])
                nc.sync.dma_start(ht_sb[:, :, :, 128:TPAD], hT.ap()[:, :, :, 128:TPAD])
            else:
                nc.sync.dma_start(ht_sb[:], hT.ap())

            # prefetch the first A chunk in 512-col pieces so the first
            # ACT unit starts as early as possible (subtile deps let the
            # first bank-slice matmuls run while later pieces stream in)
            w0 = A_WIDTHS[0]
            rt0 = rpa.tile([128, KC2, 2, A_SLOT_W[0]], FP8, tag="rta", name="rta0")
            for s in range(0, w0, 512):
                e = min(w0, s + 512)
                nc.sync.dma_start(rt0[:, :, :, s:e], eT.ap()[:, :, :, s:e])
            rt1 = None

            ones_pair = cpool.tile([128, 2, 128], FP8)
            nc.vector.memset(ones_pair[:], 1.0)
            ones_sb = cpool.tile([128, 1], F32)
            nc.vector.memset(ones_sb[:], 1.0)

            r_all = cpool.tile([128, NB, NCHA], F32)   # ACT accum slots
            s_bt = cpool.tile([128, NB], F32)          # B-lane per-token sums
            dot_sb = cpool.tile([128, NB], F32)        # exact label logits

            # B-lane accumulation target (token-replicated rows), 1 bank
            acc = pacc.tile([128, 512], F32, tag="acc", name="acc")

            # B int8 scratch ring (DVE writes, PE ones-matmul reads)
            scrB = [
                cpool.tile([128, 2, TPAD], I8, name=f"scrB{j}") for j in range(2)
            ]
            # scratch for DVE-A int32 fast-exp units
            I32 = mybir.dt.int32
            scrA = (
                cpool.tile([128, max(A_SLOT_W)], I32, name="scrA")
                if DVEA_UNITS
                else None
            )

            # late-loaded aux inputs
            wpb_sb = cpool.tile([128, NB], F32)
            id_sb = cpool.tile([128, 128], BF16)
            idg_sb = cpool.tile([128, 128], BF16)
            tscr = cpool.tile([128, 128], F32)

            def emit_acc(p, is_first, is_last):
                # vocab-dim pair-reduction over the int8 fast-exp scratch,
                # accumulated into `acc` over all pairs
                nc.tensor.matmul(
                    acc[:, :TPAD],
                    lhsT=ones_pair[:],
                    rhs=scrB[p % 2][:].bitcast(FP8),
                    perf_mode=DR,
                    start=is_first,
                    stop=is_last,
                )

            a_rt = {0: rt0}
            if rt1 is not None:
                a_rt[1] = rt1
            if WARMUP:
                wup = pbp.tile([128, 2, 128], F32, tag="ptb", name="wup")
                for _ in range(WARMUP):
                    nc.tensor.matmul(
                        wup[:, 0, :],
                        lhsT=ones_pair[:],
                        rhs=ones_pair[:],
                        perf_mode=DR,
                        start=True,
                        stop=True,
                    )
            pending_acc = None
            for ui, unit in enumerate(mixed):
                if ui == aux_at:
                    nc.sync.dma_start(wpb_sb[:], wpb.ap())
                    nc.sync.dma_start(id_sb[:], idm.ap())
                    nc.sync.dma_start(idg_sb[:], idg.ap())
                    # label-dot pairs: matmul like B pairs, diagonal via ttr
                    for p in range(NGP):
                        gt = rpb.tile(
                            [128, KC2, 2, 256], FP8, tag="rtb", name=f"rtg{p}"
                        )
                        nc.sync.dma_start(gt[:], eTG.ap()[:, p])
                        tw = min(256, TPAD - p * 256)
                        gpt = pbp.tile([128, 2, 256], F32, tag="ptb", name=f"ptg{p}")
                        for v in range(2):
                            if v * 128 >= tw:
                                continue
                            for k in range(KC2):
                                nc.tensor.matmul(
                                    gpt[:, v, :tw],
                                    lhsT=gt[:, k, :, v * 128 : (v + 1) * 128],
                                    rhs=ht_sb[:, k, :, p * 256 : p * 256 + tw],
                                    perf_mode=DR,
                                    start=(k == 0),
                                    stop=(k == KC2 - 1),
                                )
                        for v in range(2):
                            b = 2 * p + v
                            if b >= NB:
                                continue
                            nc.vector.tensor_mul(
                                tscr[:],
                                gpt[:, v, v * 128 : v * 128 + 128],
                                idg_sb[:],
                            )
                            nc.vector.tensor_reduce(
                                out=dot_sb[:, b : b + 1],
                                in_=tscr[:],
                                axis=AX.X,
                                op=add,
                            )
                if unit[0] == "B":
                    _, p, h = unit
                    HT = TPAD // 2
                    if h == 0:
                        g, gi = divmod(p, B_DMA_GROUP)
                        if gi == 0:
                            gw = min(B_DMA_GROUP, NPAIRS - p)
                            grt = rpb.tile(
                                [128, B_DMA_GROUP, KC2, 2, 256],
                                FP8,
                                tag="rtb",
                                name=f"rtb{g}",
                            )
                            nc.sync.dma_start(grt[:, :gw], eTB.ap()[:, p : p + gw])
                            b_rt = grt
                        rt_pair = b_rt[:, gi]
                    rt = rt_pair
                    pt = pbp.tile([128, 2, HT], F32, tag="ptb", name=f"ptb{p}_{h}")
                    for v in range(2):
                        for k in range(KC2):
                            nc.tensor.matmul(
                                pt[:, v, :],
                                lhsT=rt[:, k, :, v * 128 : (v + 1) * 128],
                                rhs=ht_sb[:, k, :, h * HT : (h + 1) * HT],
                                perf_mode=DR,
                                start=(k == 0),
                                stop=(k == KC2 - 1),
                            )
                    # fast-exp int8 conversion into this pair's scratch half
                    nc.vector.tensor_scalar(
                        out=scrB[p % 2][:, :, h * HT : (h + 1) * HT],
                        in0=pt[:],
                        scalar1=A8_MUL,
                        scalar2=A8_ADD,
                        op0=mult,
                        op1=add,
                    )
                    if h == 1:
                        if pending_acc is not None:
                            emit_acc(pending_acc, pending_acc == 0, False)
                        pending_acc = p
                        if p == NPAIRS - 1:  # last pair: flush immediately
                            emit_acc(p, p == 0, True)
                            pending_acc = None
                else:
                    _, ci, i = unit
                    w = A_WIDTHS[ci]
                    off = A_OFFS[ci]
                    slot = ci % 2
                    if ci not in a_rt:
                        rt = rpa.tile(
                            [128, KC2, 2, A_SLOT_W[slot]],
                            FP8,
                            tag="rta",
                            name=f"rta{ci}",
                        )
                        nc.sync.dma_start(
                            rt[:, :, :, :w], eT.ap()[:, :, :, off : off + w]
                        )
                        a_rt[ci] = rt
                    rt = a_rt[ci]
                    pool = pa0 if slot == 0 else pa1
                    pt = pool.tile(
                        [128, A_SLOT_W[slot]],
                        F32,
                        tag=f"pta{slot}",
                        name=f"pta{ci}_{i}",
                    )
                    for k in range(KC2):
                        for bk in range((w + 511) // 512):
                            s = 512 * bk
                            e = min(w, s + 512)
                            nc.tensor.matmul(
                                pt[:, s:e],
                                lhsT=ht_sb[:, k, :, i * 128 : (i + 1) * 128],
                                rhs=rt[:, k, :, s:e],
                                perf_mode=DR,
                                start=(k == 0),
                                stop=(k == KC2 - 1),
                            )
                    if (ci, i) in DVEA_UNITS:
                        # Schraudolph int32 fast exp + bitcast-f32 row sum
                        nc.vector.tensor_scalar(
                            out=scrA[:, :w],
                            in0=pt[:, :w],
                            scalar1=A32_MUL,
                            scalar2=A32_ADD,
                            op0=mult,
                            op1=add,
                        )
                        nc.vector.tensor_reduce(
                            out=r_all[:, i, ci : ci + 1],
                            in_=scrA[:, :w].bitcast(F32),
                            axis=AX.X,
                            op=add,
                        )
                    else:
                        nc.scalar.activation(
                            pt[:, :w],
                            pt[:, :w],
                            AF.Exp,
                            scale=1.0 / EMB_SCALE,
                            accum_out=r_all[:, i, ci : ci + 1],
                        )

            assert pending_acc is None

            # ---- B-lane: diagonal extraction of per-token sums --------------
            for i in range(NB):
                nc.vector.tensor_mul(
                    tscr[:], acc[:, i * 128 : (i + 1) * 128], id_sb[:]
                )
                nc.vector.tensor_reduce(
                    out=s_bt[:, i : i + 1], in_=tscr[:], axis=AX.X, op=add
                )

            # n3 columns: [sum(w*lnS) | sum(w*dot) | sum(w)] per partition
            n3 = cpool.tile([128, 3], F32)
            nc.vector.tensor_reduce(out=n3[:, 2:3], in_=wpb_sb[:], axis=AX.X, op=add)
            wdscr = cpool.tile([128, NB], F32)
            nc.vector.tensor_mul(wdscr[:], dot_sb[:], wpb_sb[:])
            nc.vector.tensor_reduce(
                out=n3[:, 1:2], in_=wdscr[:], axis=AX.X, op=add
            )

            s_sb = cpool.tile([128, NB], F32)
            nc.vector.tensor_reduce(out=s_sb[:], in_=r_all[:], axis=AX.X, op=add)
            if DEBUG_OUT:
                nc.sync.dma_start(dbg_sa.ap(), s_sb[:])
                nc.sync.dma_start(dbg_sbt.ap(), s_bt[:])
                nc.sync.dma_start(dbg_dot.ap(), dot_sb[:])
            nc.vector.tensor_add(s_sb[:], s_sb[:], s_bt[:])

            if sim_single_core:
                stot = s_sb
            else:
                cc_in = dpool.tile([128, NB], F32)
                cc_out = dpool.tile([N_CORES, 128, NB], F32, addr_space="Shared")
                nc.sync.dma_start(cc_in[:], s_sb[:])
                nc.gpsimd.collective_compute(
                    "AllGather",
                    mybir.AluOpType.bypass,
                    replica_groups=[list(range(N_CORES))],
                    ins=[cc_in.opt()],
                    outs=[cc_out.opt()],
                )
                sall = cpool.tile([128, N_CORES, NB], F32)
                nc.sync.dma_start(sall[:], cc_out.rearrange("r p i -> p r i"))
                stot = cpool.tile([128, NB], F32)
                nc.vector.tensor_add(stot[:], sall[:, 0, :], sall[:, 1, :])
                for r in range(2, N_CORES):
                    nc.vector.tensor_add(stot[:], stot[:], sall[:, r, :])

            # ---- loss = (sum(w*lnS) - sum(w*dot)) / sum(w) ------------------
            lt = cpool.tile([128, NB], F32)
            nc.scalar.activation(lt[:], stot[:], AF.Ln)
            nc.vector.tensor_mul(wdscr[:], lt[:], wpb_sb[:])
            nc.vector.tensor_reduce(
                out=n3[:, 0:1], in_=wdscr[:], axis=AX.X, op=add
            )
            # loss = (c0 - c1) / c2 is finished on the host from these sums
            ps3 = pacc.tile([1, 3], F32, tag="acc", name="ps3")
            nc.tensor.matmul(ps3[:], lhsT=ones_sb[:], rhs=n3[:], start=True, stop=True)
            ps3s = cpool.tile([1, 3], F32)
            nc.vector.tensor_copy(ps3s[:], ps3[:])
            nc.sync.dma_start(loss.ap(), ps3s[:])

    nc.compile()
    _prog_cache[key] = nc
    return nc


def prepare_in_maps(hidden, item_emb, labels_main, attention_mask, prompt_length):
    hidden = np.asarray(hidden, dtype=np.float32).reshape(B, L, D)
    item_emb = np.asarray(item_emb, dtype=np.float32).reshape(V, D)
    labels_main = np.asarray(labels_main).reshape(B, L)
    attention_mask = np.asarray(attention_mask)
    pl = int(prompt_length)

    active = attention_mask[:, pl + 1 :] == 1  # [B, L-1]
    assert active.shape == (B, L - 1), active.shape
    bb, tt = np.nonzero(active)               # row (b,t): hidden[b,t], label[b,t+1]
    n_act = len(bb)
    NB = max(1, -(-n_act // 128))
    TPAD = NB * 128

    hc = np.zeros((TPAD, D), dtype=np.float32)
    hc[:n_act] = hidden[bb, tt]
    lab = np.zeros(TPAD, dtype=np.int64)
    lab[:n_act] = np.clip(labels_main[bb, tt + 1] - LABEL_OFFSET, 0, V - 1)

    # DoubleRow layout: d = k*256 + two*128 + p  ->  [p, k, two, t]
    hT = np.ascontiguousarray(
        hc.T.reshape(KC2, 2, 128, TPAD).transpose(2, 0, 1, 3).astype(NP_FP8)
    )
    # label embedding columns in the same DR layout, pair-major like eTB
    NGP = (NB + 1) // 2
    gcols = np.zeros((D, NGP * 256), dtype=np.float32)
    gcols[:, :TPAD] = item_emb[lab].T * EMB_SCALE
    eTG = np.ascontiguousarray(
        gcols.astype(NP_FP8)
        .reshape(KC2, 2, 128, NGP, 256)
        .transpose(2, 3, 0, 1, 4)
    )  # [128, NGP, KC2, 2, 256]
    w = np.zeros((TPAD,), dtype=np.float32)
    w[:n_act] = 1.0
    wpb = np.ascontiguousarray(w.reshape(NB, 128).T)

    idm = np.eye(128, dtype=np.float32).astype(NP_BF16)
    idg = (np.eye(128, dtype=np.float32) / EMB_SCALE).astype(NP_BF16)

    emb_T = (item_emb.T * EMB_SCALE).astype(NP_FP8)  # [D, V]
    eT = np.ascontiguousarray(
        emb_T.reshape(KC2, 2, 128, V).transpose(2, 0, 1, 3)
    )  # [128, KC2, 2, V]

    in_maps = []
    for c in range(N_CORES):
        shard = eT[:, :, :, c * VS : (c + 1) * VS]
        eA = np.ascontiguousarray(shard[:, :, :, :VA])
        # pair-major B-region: [p, pair, k, two, 256]
        eB = np.ascontiguousarray(
            shard[:, :, :, VA:]
            .reshape(128, KC2, 2, NPAIRS, 256)
            .transpose(0, 3, 1, 2, 4)
        )
        in_maps.append(
            {
                "hT": hT,
                "eT": eA,
                "eTB": eB,
                "eTG": eTG,
                "wpb": wpb,
                "idm": idm,
                "idg": idg,
            }
        )
    return in_maps, NB


def kernel(hidden, item_emb, labels_main, attention_mask, prompt_length):
    in_maps, NB = prepare_in_maps(
        hidden, item_emb, labels_main, attention_mask, prompt_length
    )
    nc = build_program(NB=NB)
    last_err = None
    for _attempt in range(3):  # retry transient device/tunnel failures
        try:
            res = bass_utils.run_bass_kernel_spmd(
                nc, in_maps, core_ids=list(range(N_CORES))
            )
            c0, c1, c2 = (float(x) for x in res.results[0]["loss"][0])
            return np.float32((c0 - c1) / c2)
        except Exception as e:  # noqa: BLE001
            last_err = e
    raise last_err


# revision 74
# speedup vs baseline: 1.0045x; 1.0045x over previous
"""Fused cross-entropy loss over a 100k item vocabulary on 8 Trainium2 cores.

Math (matches the reference):
    logits = hidden_flat @ item_emb.T          # [n_tok, 100000]
    nll[r] = log(sum_v exp(logits[r, v])) - logits[r, label[r]]
    loss   = mean over ACTIVE tokens of nll

Key optimizations over a straight implementation:

1. Active-row compaction (host side): only the ~half of token rows that are
   active (attention mask past the prompt, next-token shift) contribute to
   the loss, so softmax denominators are computed only for those rows,
   gathered into NB=ceil(n_active/128) blocks of 128. Halves all device work.

2. Vocab tensor-parallel over 8 cores (12500 columns each) with fp8-e4m3
   DoubleRow matmuls (fp32 PSUM accumulate; emb pre-scaled x32 on the host).

3. Three-engine exp+row-sum. The per-core [NB*128, 12500] exp()+sum work is
   split into two vocab regions so PE, ACT and DVE all run near roofline:
     - A-region (tokens on partitions): ACT exp in place in PSUM with fused
       accumulated row-sum (accum_out).
     - B-region (vocab on partitions): DVE computes a Schraudolph-style fast
       exp via an int8 bit trick - int8(A*psum + B) IS the fp8-e4m3 bit
       pattern of ~exp(logit) - and the vocab-dim reduction is done by cheap
       PE DoubleRow ones-matmuls accumulating over all vocab pairs. The bias
       constant is tuned so the approximation is unbiased over the logit
       distribution; residual sawtooth noise (~6% per element) averages out
       over the ~39k summed terms per denominator (<0.1% on ln S).

4. Label logits ride the same PE machinery: the label embeddings are packed
   as two extra fp8 DoubleRow pairs, and each block's diagonal is extracted
   from the resulting 128x128 PSUM tiles with a (1/scale)-scaled identity
   multiply + row reduce on DVE. No extra DMA streams or DVE dot loops.

A 2KB AllGather combines per-core partial denominators; every core finishes
the masked-mean loss locally (core 0's value is returned).
"""
import sys

try:
    import concourse.bass as _cb  # provided by the environment boot path
except ModuleNotFoundError:
    sys.path.insert(0, "/opt/trn_rl_repo")

import numpy as np

import concourse.bass as bass
import concourse.bacc as bacc
import concourse.tile as tile
import concourse.mybir as mybir
from concourse import bass_utils

# Force Exp and Ln to resolve to one activation-function table set (the
# act_info set containing both) so the epilogue Ln does not pay a 1.3us
# ACT table reload on the critical tail. Indices into act_info.json are
# preserved; only membership visibility to the table-choice pass changes.
import concourse.hw_specs as _hw_specs
import concourse.bacc as _bacc_mod

_orig_get_tables = _hw_specs.get_activation_tables


def _patched_get_tables(arch):
    tabs = dict(_orig_get_tables(arch))
    AF = mybir.ActivationFunctionType
    both = [n for n, s in tabs.items() if AF.Exp in s and AF.Ln in s]
    if both:
        keep = set(both)
        tabs = {
            n: (s if n in keep else (set(s) - {AF.Exp, AF.Ln}))
            for n, s in tabs.items()
        }
    return tabs


_bacc_mod.get_activation_tables = _patched_get_tables

N_CORES = 8
B, L, D = 8, 128, 768
V = 100000
VS = V // N_CORES            # vocab shard per core
KC2 = D // 256               # DoubleRow contraction chunks
NUM_USERS = 10000
LABEL_OFFSET = 151669 + NUM_USERS

BF16 = mybir.dt.bfloat16
F32 = mybir.dt.float32
FP8 = mybir.dt.float8e4
I8 = mybir.dt.int8
NP_BF16 = mybir.dt.np(BF16)
NP_FP8 = mybir.dt.np(FP8)

EMB_SCALE = 32.0
LOG2E = 1.4426950408889634

# ---- per-core vocab split: A-region (ACT lane) | B-region (DVE+PE lane) ----
VB = 4864                    # B-region width, multiple of 256
NPAIRS = VB // 256
VA = VS - VB
# A chunk widths; chunk ci lives in PSUM slot ci%2 (bank budget per slot below)
A_SLOT_W = (1536, 1024)      # PSUM A-slot widths: 3 + 2 banks
A_WIDTHS = [512, 1024, 1536, 1024, 1536, 1024, 512, 468]
assert sum(A_WIDTHS) == VA
assert all(w <= A_SLOT_W[i % 2] for i, w in enumerate(A_WIDTHS))
# A units handed to the DVE int32 fast-exp lane instead of ACT: (chunk, block)
DVEA_UNITS = []
NCHA = len(A_WIDTHS)
A_OFFS = [sum(A_WIDTHS[:i]) for i in range(NCHA)]

# how many B half-pair units to emit before the first A unit
B_HEAD = 2
B_SPAN = 0.85                # fraction of A units over which B pairs spread
B_DMA_GROUP = 1              # B pairs fetched per DMA (pair-major eTB layout)
WARMUP = 25                  # dummy PE matmuls at t=0 to climb the p-state ramp
DEBUG_OUT = False            # dump per-row intermediates (s_a, s_bt, dot) to DRAM
HT_SPLIT = False             # split hT DMA so block-0 fills start earlier
# insert aux (hpb/gpb/w/identity) DMAs after this unit index
AUX_DMA_FRAC = 0.75

# ---------------------------------------------------------------------------
# Schraudolph fast-exp bias tuning: choose d so the estimator is unbiased
# (E[approx/true] = 1) for logits ~ N(0, sigma_l).
# ---------------------------------------------------------------------------


def _fp8e4m3_decode(i):
    i = np.asarray(i, dtype=np.int64)
    e = i >> 3
    m = i & 7
    return np.where(e > 0, (1.0 + m / 8.0) * 2.0 ** (e - 7.0), (m / 8.0) * 2.0 ** -6.0)


def _tune_d8(sigma_l=0.55, n=400000):
    # deterministic normal quantile grid
    k = (np.arange(n) + 0.5) / n
    # inverse normal CDF via numpy (Acklam-style not needed: use erfinv)
    from numpy import sqrt
    try:
        from scipy.special import erfinv  # noqa: PLC0415
        z = sqrt(2.0) * erfinv(2 * k - 1)
    except Exception:
        # logistic approximation is plenty for bias tuning
        z = np.log(k / (1 - k)) / 1.702
    y = z * sigma_l * LOG2E
    true = 2.0**y

    def bias(d):
        i = np.floor(8.0 * (y + 7.0 + d) + 0.5).astype(np.int64)
        return np.mean(_fp8e4m3_decode(i) / true) - 1.0

    lo, hi = -0.15, 0.05
    for _ in range(50):
        mid = 0.5 * (lo + hi)
        if bias(mid) > 0:
            hi = mid
        else:
            lo = mid
    return 0.5 * (lo + hi)


D8 = _tune_d8()
A8_MUL = 8.0 * LOG2E / EMB_SCALE
A8_ADD = (7.0 + D8) * 8.0


def _f32_decode(i):
    i = np.asarray(i, dtype=np.int64)
    e = i >> 23
    m = i & ((1 << 23) - 1)
    return (1.0 + m * 2.0**-23.0) * 2.0 ** (e - 127.0)


def _tune_d32(sigma_l=0.55, n=400000):
    k = (np.arange(n) + 0.5) / n
    try:
        from scipy.special import erfinv  # noqa: PLC0415
        z = np.sqrt(2.0) * erfinv(2 * k - 1)
    except Exception:
        z = np.log(k / (1 - k)) / 1.702
    y = z * sigma_l * LOG2E
    true = 2.0**y
    sc = 2.0**23

    def bias(d):
        i = np.floor(sc * (y + 127.0 + d) + 0.5).astype(np.int64)
        return np.mean(_f32_decode(i) / true) - 1.0

    lo, hi = -0.15, 0.05
    for _ in range(50):
        mid = 0.5 * (lo + hi)
        if bias(mid) > 0:
            hi = mid
        else:
            lo = mid
    return 0.5 * (lo + hi)


D32 = _tune_d32()
A32_MUL = (2.0**23) * LOG2E / EMB_SCALE
A32_ADD = (127.0 + D32) * 2.0**23

_prog_cache = {}


def _unit_schedule(NB):
    """Interleave A units (chunk-pair x block round-robin) with B pairs."""
    a_units = []
    ci = 0
    while ci < NCHA:
        pair = [ci] if ci + 1 >= NCHA else [ci, ci + 1]
        for b in range(NB):
            for c in pair:
                a_units.append(("A", c, b))
        ci += 2
    # B half-pair units: (pair, token-half); B_HEAD up front, the rest spread
    # over the first ~85% of A units
    b_units = [("B", p, h) for p in range(NPAIRS) for h in range(2)]
    nbu = len(b_units)
    mixed = list(b_units[:B_HEAD])
    rest = nbu - B_HEAD
    na = len(a_units)
    span = max(1, int(na * B_SPAN))
    next_b = B_HEAD
    for ai, au in enumerate(a_units):
        mixed.append(au)
        while next_b < nbu and (next_b - B_HEAD + 1) * span <= rest * min(ai + 1, span):
            mixed.append(b_units[next_b])
            next_b += 1
    mixed.extend(b_units[next_b:])
    return mixed


def build_program(NB: int = 4, sim_single_core: bool = False):
    key = (NB, sim_single_core)
    if key in _prog_cache:
        return _prog_cache[key]
    TPAD = NB * 128

    nc = bacc.Bacc(
        "TRN2",
        target_bir_lowering=False,
        debug=False,
        enable_asserts=True,
        num_devices=1 if sim_single_core else N_CORES,
    )
    NGP = (NB + 1) // 2  # label-embedding pairs for the PE-side label dots
    hT = nc.dram_tensor("hT", [128, KC2, 2, TPAD], FP8, kind="ExternalInput")
    eT = nc.dram_tensor("eT", [128, KC2, 2, VA], FP8, kind="ExternalInput")
    eTB = nc.dram_tensor("eTB", [128, NPAIRS, KC2, 2, 256], FP8, kind="ExternalInput")
    eTG = nc.dram_tensor("eTG", [128, NGP, KC2, 2, 256], FP8, kind="ExternalInput")
    wpb = nc.dram_tensor("wpb", [128, NB], F32, kind="ExternalInput")
    idm = nc.dram_tensor("idm", [128, 128], BF16, kind="ExternalInput")
    idg = nc.dram_tensor("idg", [128, 128], BF16, kind="ExternalInput")
    loss = nc.dram_tensor("loss", [1, 3], F32, kind="ExternalOutput")
    if DEBUG_OUT:
        dbg_sa = nc.dram_tensor("dbg_sa", [128, NB], F32, kind="ExternalOutput")
        dbg_sbt = nc.dram_tensor("dbg_sbt", [128, NB], F32, kind="ExternalOutput")
        dbg_dot = nc.dram_tensor("dbg_dot", [128, NB], F32, kind="ExternalOutput")

    add = mybir.AluOpType.add
    mult = mybir.AluOpType.mult
    AF = mybir.ActivationFunctionType
    AX = mybir.AxisListType
    DR = mybir.MatmulPerfMode.DoubleRow

    mixed = _unit_schedule(NB)
    n_units = len(mixed)
    aux_at = int(n_units * AUX_DMA_FRAC)

    with tile.TileContext(nc) as tc:
        with (
            tc.tile_pool(name="const", bufs=1) as cpool,
            tc.tile_pool(name="rta", bufs=6) as rpa,
            tc.tile_pool(name="rtb", bufs=6) as rpb,
            tc.tile_pool(name="psA0", bufs=1, space="PSUM") as pa0,
            tc.tile_pool(name="psA1", bufs=1, space="PSUM") as pa1,
            tc.tile_pool(name="psB", bufs=2, space="PSUM") as pbp,
            tc.tile_pool(name="psacc", bufs=1, space="PSUM") as pacc,
            tc.tile_pool(name="dram", bufs=1, space="DRAM") as dpool,
        ):
            # ---- resident tensors -------------------------------------------
            # block-0 token slice first: unblocks the first A fills ~1us early
            ht_sb = cpool.tile([128, KC2, 2, TPAD], FP8)
            if HT_SPLIT:
                nc.sync.dma_start(ht_sb[:, :, :, 0:128], hT.ap()[:, :, :, 0:128])
                nc.sync.dma_start(ht_sb[:, :, :, 128:TPAD], hT.ap()[:, :, :, 128:TPAD])
            else:
                nc.sync.dma_start(ht_sb[:], hT.ap())

            # prefetch the first A chunk in 512-col pieces so the first
            # ACT unit starts as early as possible (subtile deps let the
            # first bank-slice matmuls run while later pieces stream in)
            w0 = A_WIDTHS[0]
            rt0 = rpa.tile([128, KC2, 2, A_SLOT_W[0]], FP8, tag="rta", name="rta0")
            for s in range(0, w0, 512):
                e = min(w0, s + 512)
                nc.sync.dma_start(rt0[:, :, :, s:e], eT.ap()[:, :, :, s:e])
            rt1 = None

            ones_pair = cpool.tile([128, 2, 128], FP8)
            nc.vector.memset(ones_pair[:], 1.0)
            ones_sb = cpool.tile([128, 1], F32)
            nc.vector.memset(ones_sb[:], 1.0)

            r_all = cpool.tile([128, NB, NCHA], F32)   # ACT accum slots
            s_bt = cpool.tile([128, NB], F32)          # B-lane per-token sums
            dot_sb = cpool.tile([128, NB], F32)        # exact label logits

            # B-lane accumulation target (token-replicated rows), 1 bank
            acc = pacc.tile([128, 512], F32, tag="acc", name="acc")

            # B int8 scratch ring (DVE writes, PE ones-matmul reads)
            scrB = [
                cpool.tile([128, 2, TPAD], I8, name=f"scrB{j}") for j in range(2)
            ]
            # scratch for DVE-A int32 fast-exp units
            I32 = mybir.dt.int32
            scrA = (
                cpool.tile([128, max(A_SLOT_W)], I32, name="scrA")
                if DVEA_UNITS
                else None
            )

            # late-loaded aux inputs
            wpb_sb = cpool.tile([128, NB], F32)
            id_sb = cpool.tile([128, 128], BF16)
            idg_sb = cpool.tile([128, 128], BF16)
            tscr = cpool.tile([128, 128], F32)

            def emit_acc(p, is_first, is_last):
                # vocab-dim pair-reduction over the int8 fast-exp scratch,
                # accumulated into `acc` over all pairs
                nc.tensor.matmul(
                    acc[:, :TPAD],
                    lhsT=ones_pair[:],
                    rhs=scrB[p % 2][:].bitcast(FP8),
                    perf_mode=DR,
                    start=is_first,
                    stop=is_last,
                )

            a_rt = {0: rt0}
            if rt1 is not None:
                a_rt[1] = rt1
            if WARMUP:
                wup = pbp.tile([128, 2, 128], F32, tag="ptb", name="wup")
                for _ in range(WARMUP):
                    nc.tensor.matmul(
                        wup[:, 0, :],
                        lhsT=ones_pair[:],
                        rhs=ones_pair[:],
                        perf_mode=DR,
                        start=True,
                        stop=True,
                    )
            pending_acc = None
            for ui, unit in enumerate(mixed):
                if ui == aux_at:
                    nc.sync.dma_start(wpb_sb[:], wpb.ap())
                    nc.sync.dma_start(id_sb[:], idm.ap())
                    nc.sync.dma_start(idg_sb[:], idg.ap())
                    # label-dot pairs: matmul like B pairs, diagonal via ttr
                    for p in range(NGP):
                        gt = rpb.tile(
                            [128, KC2, 2, 256], FP8, tag="rtb", name=f"rtg{p}"
                        )
                        nc.sync.dma_start(gt[:], eTG.ap()[:, p])
                        tw = min(256, TPAD - p * 256)
                        gpt = pbp.tile([128, 2, 256], F32, tag="ptb", name=f"ptg{p}")
                        for v in range(2):
                            if v * 128 >= tw:
                                continue
                            for k in range(KC2):
                                nc.tensor.matmul(
                                    gpt[:, v, :tw],
                                    lhsT=gt[:, k, :, v * 128 : (v + 1) * 128],
                                    rhs=ht_sb[:, k, :, p * 256 : p * 256 + tw],
                                    perf_mode=DR,
                                    start=(k == 0),
                                    stop=(k == KC2 - 1),
                                )
                        for v in range(2):
                            b = 2 * p + v
                            if b >= NB:
                                continue
                            nc.vector.tensor_mul(
                                tscr[:],
                                gpt[:, v, v * 128 : v * 128 + 128],
                                idg_sb[:],
                            )
                            nc.vector.tensor_reduce(
                                out=dot_sb[:, b : b + 1],
                                in_=tscr[:],
                                axis=AX.X,
                                op=add,
                            )
                if unit[0] == "B":
                    _, p, h = unit
                    HT = TPAD // 2
                    if h == 0:
                        g, gi = divmod(p, B_DMA_GROUP)
                        if gi == 0:
                            gw = min(B_DMA_GROUP, NPAIRS - p)
                            grt = rpb.tile(
                                [128, B_DMA_GROUP, KC2, 2, 256],
                                FP8,
                                tag="rtb",
                                name=f"rtb{g}",
                            )
                            nc.sync.dma_start(grt[:, :gw], eTB.ap()[:, p : p + gw])
                            b_rt = grt
                        rt_pair = b_rt[:, gi]
                    rt = rt_pair
                    pt = pbp.tile([128, 2, HT], F32, tag="ptb", name=f"ptb{p}_{h}")
                    for v in range(2):
                        for k in range(KC2):
                            nc.tensor.matmul(
                                pt[:, v, :],
                                lhsT=rt[:, k, :, v * 128 : (v + 1) * 128],
                                rhs=ht_sb[:, k, :, h * HT : (h + 1) * HT],
                                perf_mode=DR,
                                start=(k == 0),
                                stop=(k == KC2 - 1),
                            )
                    # fast-exp int8 conversion into this pair's scratch half
                    nc.vector.tensor_scalar(
                        out=scrB[p % 2][:, :, h * HT : (h + 1) * HT],
                        in0=pt[:],
                        scalar1=A8_MUL,
                        scalar2=A8_ADD,
                        op0=mult,
                        op1=add,
                    )
                    if h == 1:
                        if pending_acc is not None:
                            emit_acc(pending_acc, pending_acc == 0, False)
                        pending_acc = p
                        if p == NPAIRS - 1:  # last pair: flush immediately
                            emit_acc(p, p == 0, True)
                            pending_acc = None
                else:
                    _, ci, i = unit
                    w = A_WIDTHS[ci]
                    off = A_OFFS[ci]
                    slot = ci % 2
                    if ci not in a_rt:
                        rt = rpa.tile(
                            [128, KC2, 2, A_SLOT_W[slot]],
                            FP8,
                            tag="rta",
                            name=f"rta{ci}",
                        )
                        nc.sync.dma_start(
                            rt[:, :, :, :w], eT.ap()[:, :, :, off : off + w]
                        )
                        a_rt[ci] = rt
                    rt = a_rt[ci]
                    pool = pa0 if slot == 0 else pa1
                    pt = pool.tile(
                        [128, A_SLOT_W[slot]],
                        F32,
                        tag=f"pta{slot}",
                        name=f"pta{ci}_{i}",
                    )
                    for k in range(KC2):
                        for bk in range((w + 511) // 512):
                            s = 512 * bk
                            e = min(w, s + 512)
                            nc.tensor.matmul(
                                pt[:, s:e],
                                lhsT=ht_sb[:, k, :, i * 128 : (i + 1) * 128],
                                rhs=rt[:, k, :, s:e],
                                perf_mode=DR,
                                start=(k == 0),
                                stop=(k == KC2 - 1),
                            )
                    if (ci, i) in DVEA_UNITS:
                        # Schraudolph int32 fast exp + bitcast-f32 row sum
                        nc.vector.tensor_scalar(
                            out=scrA[:, :w],
                            in0=pt[:, :w],
                            scalar1=A32_MUL,
                            scalar2=A32_ADD,
                            op0=mult,
                            op1=add,
                        )
                        nc.vector.tensor_reduce(
                            out=r_all[:, i, ci : ci + 1],
                            in_=scrA[:, :w].bitcast(F32),
                            axis=AX.X,
                            op=add,
                        )
                    else:
                        nc.scalar.activation(
                            pt[:, :w],
                            pt[:, :w],
                            AF.Exp,
                            scale=1.0 / EMB_SCALE,
                            accum_out=r_all[:, i, ci : ci + 1],
                        )

            assert pending_acc is None

            # ---- B-lane: diagonal extraction of per-token sums --------------
            for i in range(NB):
                nc.vector.tensor_mul(
                    tscr[:], acc[:, i * 128 : (i + 1) * 128], id_sb[:]
                )
                nc.vector.tensor_reduce(
                    out=s_bt[:, i : i + 1], in_=tscr[:], axis=AX.X, op=add
                )

            # n3 columns: [sum(w*lnS) | sum(w*dot) | sum(w)] per partition
            n3 = cpool.tile([128, 3], F32)
            nc.vector.tensor_reduce(out=n3[:, 2:3], in_=wpb_sb[:], axis=AX.X, op=add)
            wdscr = cpool.tile([128, NB], F32)
            nc.vector.tensor_mul(wdscr[:], dot_sb[:], wpb_sb[:])
            nc.vector.tensor_reduce(
                out=n3[:, 1:2], in_=wdscr[:], axis=AX.X, op=add
            )

            s_sb = cpool.tile([128, NB], F32)
            nc.vector.tensor_reduce(out=s_sb[:], in_=r_all[:], axis=AX.X, op=add)
            if DEBUG_OUT:
                nc.sync.dma_start(dbg_sa.ap(), s_sb[:])
                nc.sync.dma_start(dbg_sbt.ap(), s_bt[:])
                nc.sync.dma_start(dbg_dot.ap(), dot_sb[:])
            nc.vector.tensor_add(s_sb[:], s_sb[:], s_bt[:])

            if sim_single_core:
                stot = s_sb
            else:
                cc_in = dpool.tile([128, NB], F32)
                cc_out = dpool.tile([N_CORES, 128, NB], F32, addr_space="Shared")
                nc.sync.dma_start(cc_in[:], s_sb[:])
                nc.gpsimd.collective_compute(
                    "AllGather",
                    mybir.AluOpType.bypass,
                    replica_groups=[list(range(N_CORES))],
                    ins=[cc_in.opt()],
                    outs=[cc_out.opt()],
                )
                sall = cpool.tile([128, N_CORES, NB], F32)
                nc.sync.dma_start(sall[:], cc_out.rearrange("r p i -> p r i"))
                stot = cpool.tile([128, NB], F32)
                nc.vector.tensor_add(stot[:], sall[:, 0, :], sall[:, 1, :])
                for r in range(2, N_CORES):
                    nc.vector.tensor_add(stot[:], stot[:], sall[:, r, :])

            # ---- loss = (sum(w*lnS) - sum(w*dot)) / sum(w) ------------------
            lt = cpool.tile([128, NB], F32)
            nc.scalar.activation(lt[:], stot[:], AF.Ln)
            nc.vector.tensor_mul(wdscr[:], lt[:], wpb_sb[:])
            nc.vector.tensor_reduce(
                out=n3[:, 0:1], in_=wdscr[:], axis=AX.X, op=add
            )
            # loss = (c0 - c1) / c2 is finished on the host from these sums
            ps3 = pacc.tile([1, 3], F32, tag="acc", name="ps3")
            nc.tensor.matmul(ps3[:], lhsT=ones_sb[:], rhs=n3[:], start=True, stop=True)
            ps3s = cpool.tile([1, 3], F32)
            nc.vector.tensor_copy(ps3s[:], ps3[:])
            nc.sync.dma_start(loss.ap(), ps3s[:])

    nc.compile()
    _prog_cache[key] = nc
    return nc


def prepare_in_maps(hidden, item_emb, labels_main, attention_mask, prompt_length):
    hidden = np.asarray(hidden, dtype=np.float32).reshape(B, L, D)
    item_emb = np.asarray(item_emb, dtype=np.float32).reshape(V, D)
    labels_main = np.asarray(labels_main).reshape(B, L)
    attention_mask = np.asarray(attention_mask)
    pl = int(prompt_length)

    active = attention_mask[:, pl + 1 :] == 1  # [B, L-1]
    assert active.shape == (B, L - 1), active.shape
    bb, tt = np.nonzero(active)               # row (b,t): hidden[b,t], label[b,t+1]
    n_act = len(bb)
    NB = max(1, -(-n_act // 128))
    TPAD = NB * 128

    hc = np.zeros((TPAD, D), dtype=np.float32)
    hc[:n_act] = hidden[bb, tt]
    lab = np.zeros(TPAD, dtype=np.int64)
    lab[:n_act] = np.clip(labels_main[bb, tt + 1] - LABEL_OFFSET, 0, V - 1)

    # DoubleRow layout: d = k*256 + two*128 + p  ->  [p, k, two, t]
    hT = np.ascontiguousarray(
        hc.T.reshape(KC2, 2, 128, TPAD).transpose(2, 0, 1, 3).astype(NP_FP8)
    )
    # label embedding columns in the same DR layout, pair-major like eTB
    NGP = (NB + 1) // 2
    gcols = np.zeros((D, NGP * 256), dtype=np.float32)
    gcols[:, :TPAD] = item_emb[lab].T * EMB_SCALE
    eTG = np.ascontiguousarray(
        gcols.astype(NP_FP8)
        .reshape(KC2, 2, 128, NGP, 256)
        .transpose(2, 3, 0, 1, 4)
    )  # [128, NGP, KC2, 2, 256]
    w = np.zeros((TPAD,), dtype=np.float32)
    w[:n_act] = 1.0
    wpb = np.ascontiguousarray(w.reshape(NB, 128).T)

    idm = np.eye(128, dtype=np.float32).astype(NP_BF16)
    idg = (np.eye(128, dtype=np.float32) / EMB_SCALE).astype(NP_BF16)

    emb_T = (item_emb.T * EMB_SCALE).astype(NP_FP8)  # [D, V]
    eT = np.ascontiguousarray(
        emb_T.reshape(KC2, 2, 128, V).transpose(2, 0, 1, 3)
    )  # [128, KC2, 2, V]

    in_maps = []
    for c in range(N_CORES):
        shard = eT[:, :, :, c * VS : (c + 1) * VS]
        eA = np.ascontiguousarray(shard[:, :, :, :VA])
        # pair-major B-region: [p, pair, k, two, 256]
        eB = np.ascontiguousarray(
            shard[:, :, :, VA:]
            .reshape(128, KC2, 2, NPAIRS, 256)
            .transpose(0, 3, 1, 2, 4)
        )
        in_maps.append(
            {
                "hT": hT,
                "eT": eA,
                "eTB": eB,
                "eTG": eTG,
                "wpb": wpb,
                "idm": idm,
                "idg": idg,
            }
        )
    return in_maps, NB


def kernel(hidden, item_emb, labels_main, attention_mask, prompt_length):
    in_maps, NB = prepare_in_maps(
        hidden, item_emb, labels_main, attention_mask, prompt_length
    )
    nc = build_program(NB=NB)
    last_err = None
    for _attempt in range(3):  # retry transient device/tunnel failures
        try:
            res = bass_utils.run_bass_kernel_spmd(
                nc, in_maps, core_ids=list(range(N_CORES))
            )
            c0, c1, c2 = (float(x) for x in res.results[0]["loss"][0])
            return np.float32((c0 - c1) / c2)
        except Exception as e:  # noqa: BLE001
            last_err = e
    raise last_err


# revision 75
# speedup vs baseline: 1.0113x; 1.0068x over previous
"""Fused cross-entropy loss over a 100k item vocabulary on 8 Trainium2 cores.

Math (matches the reference):
    logits = hidden_flat @ item_emb.T          # [n_tok, 100000]
    nll[r] = log(sum_v exp(logits[r, v])) - logits[r, label[r]]
    loss   = mean over ACTIVE tokens of nll

Key optimizations over a straight implementation:

1. Active-row compaction (host side): only the ~half of token rows that are
   active (attention mask past the prompt, next-token shift) contribute to
   the loss, so softmax denominators are computed only for those rows,
   gathered into NB=ceil(n_active/128) blocks of 128. Halves all device work.

2. Vocab tensor-parallel over 8 cores (12500 columns each) with fp8-e4m3
   DoubleRow matmuls (fp32 PSUM accumulate; emb pre-scaled x32 on the host).

3. Three-engine exp+row-sum. The per-core [NB*128, 12500] exp()+sum work is
   split into two vocab regions so PE, ACT and DVE all run near roofline:
     - A-region (tokens on partitions): ACT exp in place in PSUM with fused
       accumulated row-sum (accum_out).
     - B-region (vocab on partitions): DVE computes a Schraudolph-style fast
       exp via an int8 bit trick - int8(A*psum + B) IS the fp8-e4m3 bit
       pattern of ~exp(logit) - and the vocab-dim reduction is done by cheap
       PE DoubleRow ones-matmuls accumulating over all vocab pairs. The bias
       constant is tuned so the approximation is unbiased over the logit
       distribution; residual sawtooth noise (~6% per element) averages out
       over the ~39k summed terms per denominator (<0.1% on ln S).

4. Label logits ride the same PE machinery: the label embeddings are packed
   as two extra fp8 DoubleRow pairs, and each block's diagonal is extracted
   from the resulting 128x128 PSUM tiles with a (1/scale)-scaled identity
   multiply + row reduce on DVE. No extra DMA streams or DVE dot loops.

A 2KB AllGather combines per-core partial denominators; every core finishes
the masked-mean loss locally (core 0's value is returned).
"""
import sys

try:
    import concourse.bass as _cb  # provided by the environment boot path
except ModuleNotFoundError:
    sys.path.insert(0, "/opt/trn_rl_repo")

import numpy as np

import concourse.bass as bass
import concourse.bacc as bacc
import concourse.tile as tile
import concourse.mybir as mybir
from concourse import bass_utils

# Force Exp and Ln to resolve to one activation-function table set (the
# act_info set containing both) so the epilogue Ln does not pay a 1.3us
# ACT table reload on the critical tail. Indices into act_info.json are
# preserved; only membership visibility to the table-choice pass changes.
import concourse.hw_specs as _hw_specs
import concourse.bacc as _bacc_mod

_orig_get_tables = _hw_specs.get_activation_tables


def _patched_get_tables(arch):
    tabs = dict(_orig_get_tables(arch))
    AF = mybir.ActivationFunctionType
    both = [n for n, s in tabs.items() if AF.Exp in s and AF.Ln in s]
    if both:
        keep = set(both)
        tabs = {
            n: (s if n in keep else (set(s) - {AF.Exp, AF.Ln}))
            for n, s in tabs.items()
        }
    return tabs


_bacc_mod.get_activation_tables = _patched_get_tables

N_CORES = 8
B, L, D = 8, 128, 768
V = 100000
VS = V // N_CORES            # vocab shard per core
KC2 = D // 256               # DoubleRow contraction chunks
NUM_USERS = 10000
LABEL_OFFSET = 151669 + NUM_USERS

BF16 = mybir.dt.bfloat16
F32 = mybir.dt.float32
FP8 = mybir.dt.float8e4
I8 = mybir.dt.int8
NP_BF16 = mybir.dt.np(BF16)
NP_FP8 = mybir.dt.np(FP8)

EMB_SCALE = 32.0
LOG2E = 1.4426950408889634

# ---- per-core vocab split: A-region (ACT lane) | B-region (DVE+PE lane) ----
VB = 4864                    # B-region width, multiple of 256
NPAIRS = VB // 256
VA = VS - VB
# A chunk widths; chunk ci lives in PSUM slot ci%2 (bank budget per slot below)
A_SLOT_W = (1536, 1024)      # PSUM A-slot widths: 3 + 2 banks
A_WIDTHS = [512, 1024, 1536, 1024, 1536, 1024, 512, 468]
assert sum(A_WIDTHS) == VA
assert all(w <= A_SLOT_W[i % 2] for i, w in enumerate(A_WIDTHS))
# A units handed to the DVE int32 fast-exp lane instead of ACT: (chunk, block)
DVEA_UNITS = []
NCHA = len(A_WIDTHS)
A_OFFS = [sum(A_WIDTHS[:i]) for i in range(NCHA)]

# how many B half-pair units to emit before the first A unit
B_HEAD = 2
B_SPAN = 0.80                # fraction of A units over which B pairs spread
B_DMA_GROUP = 1              # B pairs fetched per DMA (pair-major eTB layout)
WARMUP = 25                  # dummy PE matmuls at t=0 to climb the p-state ramp
DEBUG_OUT = False            # dump per-row intermediates (s_a, s_bt, dot) to DRAM
HT_SPLIT = False             # split hT DMA so block-0 fills start earlier
# insert aux (hpb/gpb/w/identity) DMAs after this unit index
AUX_DMA_FRAC = 0.75

# ---------------------------------------------------------------------------
# Schraudolph fast-exp bias tuning: choose d so the estimator is unbiased
# (E[approx/true] = 1) for logits ~ N(0, sigma_l).
# ---------------------------------------------------------------------------


def _fp8e4m3_decode(i):
    i = np.asarray(i, dtype=np.int64)
    e = i >> 3
    m = i & 7
    return np.where(e > 0, (1.0 + m / 8.0) * 2.0 ** (e - 7.0), (m / 8.0) * 2.0 ** -6.0)


def _tune_d8(sigma_l=0.55, n=400000):
    # deterministic normal quantile grid
    k = (np.arange(n) + 0.5) / n
    # inverse normal CDF via numpy (Acklam-style not needed: use erfinv)
    from numpy import sqrt
    try:
        from scipy.special import erfinv  # noqa: PLC0415
        z = sqrt(2.0) * erfinv(2 * k - 1)
    except Exception:
        # logistic approximation is plenty for bias tuning
        z = np.log(k / (1 - k)) / 1.702
    y = z * sigma_l * LOG2E
    true = 2.0**y

    def bias(d):
        i = np.floor(8.0 * (y + 7.0 + d) + 0.5).astype(np.int64)
        return np.mean(_fp8e4m3_decode(i) / true) - 1.0

    lo, hi = -0.15, 0.05
    for _ in range(50):
        mid = 0.5 * (lo + hi)
        if bias(mid) > 0:
            hi = mid
        else:
            lo = mid
    return 0.5 * (lo + hi)


D8 = _tune_d8()
A8_MUL = 8.0 * LOG2E / EMB_SCALE
A8_ADD = (7.0 + D8) * 8.0


def _f32_decode(i):
    i = np.asarray(i, dtype=np.int64)
    e = i >> 23
    m = i & ((1 << 23) - 1)
    return (1.0 + m * 2.0**-23.0) * 2.0 ** (e - 127.0)


def _tune_d32(sigma_l=0.55, n=400000):
    k = (np.arange(n) + 0.5) / n
    try:
        from scipy.special import erfinv  # noqa: PLC0415
        z = np.sqrt(2.0) * erfinv(2 * k - 1)
    except Exception:
        z = np.log(k / (1 - k)) / 1.702
    y = z * sigma_l * LOG2E
    true = 2.0**y
    sc = 2.0**23

    def bias(d):
        i = np.floor(sc * (y + 127.0 + d) + 0.5).astype(np.int64)
        return np.mean(_f32_decode(i) / true) - 1.0

    lo, hi = -0.15, 0.05
    for _ in range(50):
        mid = 0.5 * (lo + hi)
        if bias(mid) > 0:
            hi = mid
        else:
            lo = mid
    return 0.5 * (lo + hi)


D32 = _tune_d32()
A32_MUL = (2.0**23) * LOG2E / EMB_SCALE
A32_ADD = (127.0 + D32) * 2.0**23

_prog_cache = {}


def _unit_schedule(NB):
    """Interleave A units (chunk-pair x block round-robin) with B pairs."""
    a_units = []
    ci = 0
    while ci < NCHA:
        pair = [ci] if ci + 1 >= NCHA else [ci, ci + 1]
        for b in range(NB):
            for c in pair:
                a_units.append(("A", c, b))
        ci += 2
    # B half-pair units: (pair, token-half); B_HEAD up front, the rest spread
    # over the first ~85% of A units
    b_units = [("B", p, h) for p in range(NPAIRS) for h in range(2)]
    nbu = len(b_units)
    mixed = list(b_units[:B_HEAD])
    rest = nbu - B_HEAD
    na = len(a_units)
    span = max(1, int(na * B_SPAN))
    next_b = B_HEAD
    for ai, au in enumerate(a_units):
        mixed.append(au)
        while next_b < nbu and (next_b - B_HEAD + 1) * span <= rest * min(ai + 1, span):
            mixed.append(b_units[next_b])
            next_b += 1
    mixed.extend(b_units[next_b:])
    return mixed


def build_program(NB: int = 4, sim_single_core: bool = False):
    key = (NB, sim_single_core)
    if key in _prog_cache:
        return _prog_cache[key]
    TPAD = NB * 128

    nc = bacc.Bacc(
        "TRN2",
        target_bir_lowering=False,
        debug=False,
        enable_asserts=True,
        num_devices=1 if sim_single_core else N_CORES,
    )
    NGP = (NB + 1) // 2  # label-embedding pairs for the PE-side label dots
    hT = nc.dram_tensor("hT", [128, KC2, 2, TPAD], FP8, kind="ExternalInput")
    eT = nc.dram_tensor("eT", [128, KC2, 2, VA], FP8, kind="ExternalInput")
    eTB = nc.dram_tensor("eTB", [128, NPAIRS, KC2, 2, 256], FP8, kind="ExternalInput")
    eTG = nc.dram_tensor("eTG", [128, NGP, KC2, 2, 256], FP8, kind="ExternalInput")
    wpb = nc.dram_tensor("wpb", [128, NB], F32, kind="ExternalInput")
    idm = nc.dram_tensor("idm", [128, 128], BF16, kind="ExternalInput")
    idg = nc.dram_tensor("idg", [128, 128], BF16, kind="ExternalInput")
    loss = nc.dram_tensor("loss", [1, 3], F32, kind="ExternalOutput")
    if DEBUG_OUT:
        dbg_sa = nc.dram_tensor("dbg_sa", [128, NB], F32, kind="ExternalOutput")
        dbg_sbt = nc.dram_tensor("dbg_sbt", [128, NB], F32, kind="ExternalOutput")
        dbg_dot = nc.dram_tensor("dbg_dot", [128, NB], F32, kind="ExternalOutput")

    add = mybir.AluOpType.add
    mult = mybir.AluOpType.mult
    AF = mybir.ActivationFunctionType
    AX = mybir.AxisListType
    DR = mybir.MatmulPerfMode.DoubleRow

    mixed = _unit_schedule(NB)
    n_units = len(mixed)
    aux_at = int(n_units * AUX_DMA_FRAC)

    with tile.TileContext(nc) as tc:
        with (
            tc.tile_pool(name="const", bufs=1) as cpool,
            tc.tile_pool(name="rta", bufs=6) as rpa,
            tc.tile_pool(name="rtb", bufs=6) as rpb,
            tc.tile_pool(name="psA0", bufs=1, space="PSUM") as pa0,
            tc.tile_pool(name="psA1", bufs=1, space="PSUM") as pa1,
            tc.tile_pool(name="psB", bufs=2, space="PSUM") as pbp,
            tc.tile_pool(name="psacc", bufs=1, space="PSUM") as pacc,
            tc.tile_pool(name="dram", bufs=1, space="DRAM") as dpool,
        ):
            # ---- resident tensors -------------------------------------------
            # block-0 token slice first: unblocks the first A fills ~1us early
            ht_sb = cpool.tile([128, KC2, 2, TPAD], FP8)
            if HT_SPLIT:
                nc.sync.dma_start(ht_sb[:, :, :, 0:128], hT.ap()[:, :, :, 0:128])
                nc.sync.dma_start(ht_sb[:, :, :, 128:TPAD], hT.ap()[:, :, :, 128:TPAD])
            else:
                nc.sync.dma_start(ht_sb[:], hT.ap())

            # prefetch the first A chunk in 512-col pieces so the first
            # ACT unit starts as early as possible (subtile deps let the
            # first bank-slice matmuls run while later pieces stream in)
            w0 = A_WIDTHS[0]
            rt0 = rpa.tile([128, KC2, 2, A_SLOT_W[0]], FP8, tag="rta", name="rta0")
            for s in range(0, w0, 512):
                e = min(w0, s + 512)
                nc.sync.dma_start(rt0[:, :, :, s:e], eT.ap()[:, :, :, s:e])
            rt1 = None

            ones_pair = cpool.tile([128, 2, 128], FP8)
            nc.vector.memset(ones_pair[:], 1.0)
            ones_sb = cpool.tile([128, 1], F32)
            nc.vector.memset(ones_sb[:], 1.0)

            r_all = cpool.tile([128, NB, NCHA], F32)   # ACT accum slots
            s_bt = cpool.tile([128, NB], F32)          # B-lane per-token sums
            dot_sb = cpool.tile([128, NB], F32)        # exact label logits

            # B-lane accumulation target (token-replicated rows), 1 bank
            acc = pacc.tile([128, 512], F32, tag="acc", name="acc")

            # B int8 scratch ring (DVE writes, PE ones-matmul reads)
            scrB = [
                cpool.tile([128, 2, TPAD], I8, name=f"scrB{j}") for j in range(2)
            ]
            # scratch for DVE-A int32 fast-exp units
            I32 = mybir.dt.int32
            scrA = (
                cpool.tile([128, max(A_SLOT_W)], I32, name="scrA")
                if DVEA_UNITS
                else None
            )

            # late-loaded aux inputs
            wpb_sb = cpool.tile([128, NB], F32)
            id_sb = cpool.tile([128, 128], BF16)
            idg_sb = cpool.tile([128, 128], BF16)
            tscr = cpool.tile([128, 128], F32)

            def emit_acc(p, is_first, is_last):
                # vocab-dim pair-reduction over the int8 fast-exp scratch,
                # accumulated into `acc` over all pairs
                nc.tensor.matmul(
                    acc[:, :TPAD],
                    lhsT=ones_pair[:],
                    rhs=scrB[p % 2][:].bitcast(FP8),
                    perf_mode=DR,
                    start=is_first,
                    stop=is_last,
                )

            a_rt = {0: rt0}
            if rt1 is not None:
                a_rt[1] = rt1
            if WARMUP:
                wup = pbp.tile([128, 2, 128], F32, tag="ptb", name="wup")
                for _ in range(WARMUP):
                    nc.tensor.matmul(
                        wup[:, 0, :],
                        lhsT=ones_pair[:],
                        rhs=ones_pair[:],
                        perf_mode=DR,
                        start=True,
                        stop=True,
                    )
            pending_acc = None
            for ui, unit in enumerate(mixed):
                if ui == aux_at:
                    nc.sync.dma_start(wpb_sb[:], wpb.ap())
                    nc.sync.dma_start(id_sb[:], idm.ap())
                    nc.sync.dma_start(idg_sb[:], idg.ap())
                    # label-dot pairs: matmul like B pairs, diagonal via ttr
                    for p in range(NGP):
                        gt = rpb.tile(
                            [128, KC2, 2, 256], FP8, tag="rtb", name=f"rtg{p}"
                        )
                        nc.sync.dma_start(gt[:], eTG.ap()[:, p])
                        tw = min(256, TPAD - p * 256)
                        gpt = pbp.tile([128, 2, 256], F32, tag="ptb", name=f"ptg{p}")
                        for v in range(2):
                            if v * 128 >= tw:
                                continue
                            for k in range(KC2):
                                nc.tensor.matmul(
                                    gpt[:, v, :tw],
                                    lhsT=gt[:, k, :, v * 128 : (v + 1) * 128],
                                    rhs=ht_sb[:, k, :, p * 256 : p * 256 + tw],
                                    perf_mode=DR,
                                    start=(k == 0),
                                    stop=(k == KC2 - 1),
                                )
                        for v in range(2):
                            b = 2 * p + v
                            if b >= NB:
                                continue
                            nc.vector.tensor_mul(
                                tscr[:],
                                gpt[:, v, v * 128 : v * 128 + 128],
                                idg_sb[:],
                            )
                            nc.vector.tensor_reduce(
                                out=dot_sb[:, b : b + 1],
                                in_=tscr[:],
                                axis=AX.X,
                                op=add,
                            )
                if unit[0] == "B":
                    _, p, h = unit
                    HT = TPAD // 2
                    if h == 0:
                        g, gi = divmod(p, B_DMA_GROUP)
                        if gi == 0:
                            gw = min(B_DMA_GROUP, NPAIRS - p)
                            grt = rpb.tile(
                                [128, B_DMA_GROUP, KC2, 2, 256],
                                FP8,
                                tag="rtb",
                                name=f"rtb{g}",
                            )
                            nc.sync.dma_start(grt[:, :gw], eTB.ap()[:, p : p + gw])
                            b_rt = grt
                        rt_pair = b_rt[:, gi]
                    rt = rt_pair
                    pt = pbp.tile([128, 2, HT], F32, tag="ptb", name=f"ptb{p}_{h}")
                    for v in range(2):
                        for k in range(KC2):
                            nc.tensor.matmul(
                                pt[:, v, :],
                                lhsT=rt[:, k, :, v * 128 : (v + 1) * 128],
                                rhs=ht_sb[:, k, :, h * HT : (h + 1) * HT],
                                perf_mode=DR,
                                start=(k == 0),
                                stop=(k == KC2 - 1),
                            )
                    # fast-exp int8 conversion into this pair's scratch half
                    nc.vector.tensor_scalar(
                        out=scrB[p % 2][:, :, h * HT : (h + 1) * HT],
                        in0=pt[:],
                        scalar1=A8_MUL,
                        scalar2=A8_ADD,
                        op0=mult,
                        op1=add,
                    )
                    if h == 1:
                        if pending_acc is not None:
                            emit_acc(pending_acc, pending_acc == 0, False)
                        pending_acc = p
                        if p == NPAIRS - 1:  # last pair: flush immediately
                            emit_acc(p, p == 0, True)
                            pending_acc = None
                else:
                    _, ci, i = unit
                    w = A_WIDTHS[ci]
                    off = A_OFFS[ci]
                    slot = ci % 2
                    if ci not in a_rt:
                        rt = rpa.tile(
                            [128, KC2, 2, A_SLOT_W[slot]],
                            FP8,
                            tag="rta",
                            name=f"rta{ci}",
                        )
                        nc.sync.dma_start(
                            rt[:, :, :, :w], eT.ap()[:, :, :, off : off + w]
                        )
                        a_rt[ci] = rt
                    rt = a_rt[ci]
                    pool = pa0 if slot == 0 else pa1
                    pt = pool.tile(
                        [128, A_SLOT_W[slot]],
                        F32,
                        tag=f"pta{slot}",
                        name=f"pta{ci}_{i}",
                    )
                    for k in range(KC2):
                        for bk in range((w + 511) // 512):
                            s = 512 * bk
                            e = min(w, s + 512)
                            nc.tensor.matmul(
                                pt[:, s:e],
                                lhsT=ht_sb[:, k, :, i * 128 : (i + 1) * 128],
                                rhs=rt[:, k, :, s:e],
                                perf_mode=DR,
                                start=(k == 0),
                                stop=(k == KC2 - 1),
                            )
                    if (ci, i) in DVEA_UNITS:
                        # Schraudolph int32 fast exp + bitcast-f32 row sum
                        nc.vector.tensor_scalar(
                            out=scrA[:, :w],
                            in0=pt[:, :w],
                            scalar1=A32_MUL,
                            scalar2=A32_ADD,
                            op0=mult,
                            op1=add,
                        )
                        nc.vector.tensor_reduce(
                            out=r_all[:, i, ci : ci + 1],
                            in_=scrA[:, :w].bitcast(F32),
                            axis=AX.X,
                            op=add,
                        )
                    else:
                        nc.scalar.activation(
                            pt[:, :w],
                            pt[:, :w],
                            AF.Exp,
                            scale=1.0 / EMB_SCALE,
                            accum_out=r_all[:, i, ci : ci + 1],
                        )

            assert pending_acc is None

            # ---- B-lane: diagonal extraction of per-token sums --------------
            for i in range(NB):
                nc.vector.tensor_mul(
                    tscr[:], acc[:, i * 128 : (i + 1) * 128], id_sb[:]
                )
                nc.vector.tensor_reduce(
                    out=s_bt[:, i : i + 1], in_=tscr[:], axis=AX.X, op=add
                )

            # n3 columns: [sum(w*lnS) | sum(w*dot) | sum(w)] per partition
            n3 = cpool.tile([128, 3], F32)
            nc.vector.tensor_reduce(out=n3[:, 2:3], in_=wpb_sb[:], axis=AX.X, op=add)
            wdscr = cpool.tile([128, NB], F32)
            nc.vector.tensor_mul(wdscr[:], dot_sb[:], wpb_sb[:])
            nc.vector.tensor_reduce(
                out=n3[:, 1:2], in_=wdscr[:], axis=AX.X, op=add
            )

            s_sb = cpool.tile([128, NB], F32)
            nc.vector.tensor_reduce(out=s_sb[:], in_=r_all[:], axis=AX.X, op=add)
            if DEBUG_OUT:
                nc.sync.dma_start(dbg_sa.ap(), s_sb[:])
                nc.sync.dma_start(dbg_sbt.ap(), s_bt[:])
                nc.sync.dma_start(dbg_dot.ap(), dot_sb[:])
            nc.vector.tensor_add(s_sb[:], s_sb[:], s_bt[:])

            if sim_single_core:
                stot = s_sb
            else:
                cc_in = dpool.tile([128, NB], F32)
                cc_out = dpool.tile([N_CORES, 128, NB], F32, addr_space="Shared")
                nc.sync.dma_start(cc_in[:], s_sb[:])
                nc.gpsimd.collective_compute(
                    "AllGather",
                    mybir.AluOpType.bypass,
                    replica_groups=[list(range(N_CORES))],
                    ins=[cc_in.opt()],
                    outs=[cc_out.opt()],
                )
                sall = cpool.tile([128, N_CORES, NB], F32)
                nc.sync.dma_start(sall[:], cc_out.rearrange("r p i -> p r i"))
                stot = cpool.tile([128, NB], F32)
                nc.vector.tensor_add(stot[:], sall[:, 0, :], sall[:, 1, :])
                for r in range(2, N_CORES):
                    nc.vector.tensor_add(stot[:], stot[:], sall[:, r, :])

            # ---- loss = (sum(w*lnS) - sum(w*dot)) / sum(w) ------------------
            lt = cpool.tile([128, NB], F32)
            nc.scalar.activation(lt[:], stot[:], AF.Ln)
            nc.vector.tensor_mul(wdscr[:], lt[:], wpb_sb[:])
            nc.vector.tensor_reduce(
                out=n3[:, 0:1], in_=wdscr[:], axis=AX.X, op=add
            )
            # loss = (c0 - c1) / c2 is finished on the host from these sums
            ps3 = pacc.tile([1, 3], F32, tag="acc", name="ps3")
            nc.tensor.matmul(ps3[:], lhsT=ones_sb[:], rhs=n3[:], start=True, stop=True)
            ps3s = cpool.tile([1, 3], F32)
            nc.vector.tensor_copy(ps3s[:], ps3[:])
            nc.sync.dma_start(loss.ap(), ps3s[:])

    nc.compile()
    _prog_cache[key] = nc
    return nc


def prepare_in_maps(hidden, item_emb, labels_main, attention_mask, prompt_length):
    hidden = np.asarray(hidden, dtype=np.float32).reshape(B, L, D)
    item_emb = np.asarray(item_emb, dtype=np.float32).reshape(V, D)
    labels_main = np.asarray(labels_main).reshape(B, L)
    attention_mask = np.asarray(attention_mask)
    pl = int(prompt_length)

    active = attention_mask[:, pl + 1 :] == 1  # [B, L-1]
    assert active.shape == (B, L - 1), active.shape
    bb, tt = np.nonzero(active)               # row (b,t): hidden[b,t], label[b,t+1]
    n_act = len(bb)
    NB = max(1, -(-n_act // 128))
    TPAD = NB * 128

    hc = np.zeros((TPAD, D), dtype=np.float32)
    hc[:n_act] = hidden[bb, tt]
    lab = np.zeros(TPAD, dtype=np.int64)
    lab[:n_act] = np.clip(labels_main[bb, tt + 1] - LABEL_OFFSET, 0, V - 1)

    # DoubleRow layout: d = k*256 + two*128 + p  ->  [p, k, two, t]
    hT = np.ascontiguousarray(
        hc.T.reshape(KC2, 2, 128, TPAD).transpose(2, 0, 1, 3).astype(NP_FP8)
    )
    # label embedding columns in the same DR layout, pair-major like eTB
    NGP = (NB + 1) // 2
    gcols = np.zeros((D, NGP * 256), dtype=np.float32)
    gcols[:, :TPAD] = item_emb[lab].T * EMB_SCALE
    eTG = np.ascontiguousarray(
        gcols.astype(NP_FP8)
        .reshape(KC2, 2, 128, NGP, 256)
        .transpose(2, 3, 0, 1, 4)
    )  # [128, NGP, KC2, 2, 256]
    w = np.zeros((TPAD,), dtype=np.float32)
    w[:n_act] = 1.0
    wpb = np.ascontiguousarray(w.reshape(NB, 128).T)

    idm = np.eye(128, dtype=np.float32).astype(NP_BF16)
    idg = (np.eye(128, dtype=np.float32) / EMB_SCALE).astype(NP_BF16)

    emb_T = (item_emb.T * EMB_SCALE).astype(NP_FP8)  # [D, V]
    eT = np.ascontiguousarray(
        emb_T.reshape(KC2, 2, 128, V).transpose(2, 0, 1, 3)
    )  # [128, KC2, 2, V]

    in_maps = []
    for c in range(N_CORES):
        shard = eT[:, :, :, c * VS : (c + 1) * VS]
        eA = np.ascontiguousarray(shard[:, :, :, :VA])
        # pair-major B-region: [p, pair, k, two, 256]
        eB = np.ascontiguousarray(
            shard[:, :, :, VA:]
            .reshape(128, KC2, 2, NPAIRS, 256)
            .transpose(0, 3, 1, 2, 4)
        )
        in_maps.append(
            {
                "hT": hT,
                "eT": eA,
                "eTB": eB,
                "eTG": eTG,
                "wpb": wpb,
                "idm": idm,
                "idg": idg,
            }
        )
    return in_maps, NB


def kernel(hidden, item_emb, labels_main, attention_mask, prompt_length):
    in_maps, NB = prepare_in_maps(
        hidden, item_emb, labels_main, attention_mask, prompt_length
    )
    nc = build_program(NB=NB)
    last_err = None
    for _attempt in range(3):  # retry transient device/tunnel failures
        try:
            res = bass_utils.run_bass_kernel_spmd(
                nc, in_maps, core_ids=list(range(N_CORES))
            )
            c0, c1, c2 = (float(x) for x in res.results[0]["loss"][0])
            return np.float32((c0 - c1) / c2)
        except Exception as e:  # noqa: BLE001
            last_err = e
    raise last_err
